# revision 36
# baseline (speedup 1.0000x reference)
"""DeepSets (MLP + ragged segment-mean) Trainium2 Bass kernel.

Full inputs in / full outputs out. Internally: data-parallel over sets --
tokens are sharded by contiguous whole-segment ranges across 8 NeuronCores
(balanced by token count), the tiny MLP weights are replicated, and the
segment-mean is fully local per core.

Per-core device pipeline (feature-major, x pre-transposed + bf16 on host):
  L1  : psum_h1[dh,t] = W1.T @ xT            (TensorE bf16, 2 psum tiles)
  relu: h1 = relu(psum_h1 + b1) -> sbuf bf16 (ACT / DVE, split for balance)
  L2  : psum_h2[f,t] = W2.T @ h1             (TensorE bf16, 2-chunk accum)
  relu: h2 = relu(psum_h2 + b2) -> sbuf bf16 (ACT / DVE, split for balance)
  scan: win[f,1+t] = cumsum of h2 cols       (DVE / GpSimd alternating,
        fp32 state, chained across windows via `initial` -- no resets)
  gath: g[slot] = win at segment-end cols    (GpSimd ap_gather, host idx)
  out : diff 128-slot tiles -> transpose (TensorE) -> scale 1/count -> DMA
"""

import math
from contextlib import ExitStack

import ml_dtypes
import numpy as np

import concourse.bass as bass
import concourse.tile as tile
from concourse import bacc, mybir
from concourse.bass_utils import run_bass_kernel_spmd

N_CORES = 8
D_IN, D_H, D_OUT = 128, 256, 128
WIN = 2048  # tokens per scan/gather window
ITER = 512  # tokens per MLP pipeline iteration (one psum bank)
SBUF_BUFS = 3
# engine schedule (tuned on the sim): h1a and h2r evacs on ACT, h1b on DVE
# except one pair per 4 windows on ACT; scans are DVE-only (the scan opcode
# is illegal on the Pool/GpSimd engine -- codegen ISA check rejects it).
H1B_ACT_EVERY = 3  # windows between h1b pairs handed to ACT (0 = never)

F32 = mybir.dt.float32
F32R = mybir.dt.float32r
BF16 = mybir.dt.bfloat16
I16 = mybir.dt.int16
RELU = mybir.ActivationFunctionType.Relu
ADD = mybir.AluOpType.add
SUB = mybir.AluOpType.subtract
MULT = mybir.AluOpType.mult
MAX = mybir.AluOpType.max


def _window_lengths(t_pad):
    """Full windows of WIN plus one partial tail window (multiple of 16)."""
    lens = [WIN] * (t_pad // WIN)
    if t_pad % WIN:
        lens.append(t_pad % WIN)
    return lens


def _build_program(t_pad: int, spw: int, n_tr: int, reps: int = 1, mode: str = "full"):
    """Build the single-core SPMD program for t_pad tokens per core.

    spw: gather slots per window (multiple of 16)
    n_tr: number of 128-slot output tiles (out rows = n_tr*128)
    reps: execute the whole pipeline this many times (timing use only)
    mode: "full" | "mlp" (skip scan+gather+epilogue) | "scan" (skip
          gather+epilogue) -- hardware timing ablations; wrong results
    """
    wlens = _window_lengths(t_pad)
    n_win = len(wlens)
    spw16 = spw // 16
    # idx blocks padded to 8 int16 columns (16B) so each window's slice is
    # cacheline-aligned -- GpSimd misreads 2-byte-misaligned idx slices
    idxp = ((spw16 + 7) // 8) * 8
    g_len = n_tr * 128

    nc = bacc.Bacc(
        "TRN2", target_bir_lowering=False, debug=False, num_devices=N_CORES
    )
    xT = nc.dram_tensor("xT", [D_IN, t_pad], BF16, kind="ExternalInput").ap()
    w1 = nc.dram_tensor("w1", [D_IN, D_H], BF16, kind="ExternalInput").ap()
    # w2 packed on host: [:, 0:128] = W2[0:128,:], [:, 128:256] = W2[128:256,:]
    w2 = nc.dram_tensor("w2", [128, 2 * D_OUT], BF16, kind="ExternalInput").ap()
    b1 = nc.dram_tensor("b1", [128, 2], F32, kind="ExternalInput").ap()
    b2 = nc.dram_tensor("b2", [128, 1], F32, kind="ExternalInput").ap()
    eye = nc.dram_tensor("eye", [128, 128], F32, kind="ExternalInput").ap()
    gidx = nc.dram_tensor("gidx", [128, n_win * idxp], I16, kind="ExternalInput").ap()
    invc = nc.dram_tensor("invc", [128, n_tr], F32, kind="ExternalInput").ap()
    out = nc.dram_tensor("out", [g_len, D_OUT], F32, kind="ExternalOutput").ap()

    with tile.TileContext(nc) as tc, ExitStack() as ctx:
        singles = ctx.enter_context(tc.tile_pool(name="singles", bufs=1))
        xin = ctx.enter_context(tc.tile_pool(name="xin", bufs=SBUF_BUFS))
        h1sb = ctx.enter_context(tc.tile_pool(name="h1sb", bufs=SBUF_BUFS))
        h2wp = ctx.enter_context(tc.tile_pool(name="h2wp", bufs=3))
        winp = ctx.enter_context(tc.tile_pool(name="winp", bufs=3))
        gp = ctx.enter_context(tc.tile_pool(name="gp", bufs=1))
        outp = ctx.enter_context(tc.tile_pool(name="outp", bufs=2))
        totp = ctx.enter_context(tc.tile_pool(name="totp", bufs=2))
        # h1 psum: per-iter single-bank tiles, double-buffered (keeps the PE
        # pipeline free-running). h2 psum: one [128, 2*ITER] tile spanning two
        # banks so the relu/evac drains an iteration pair in a single wide op
        # (same bias across the pair); its short chain hides single-buffering.
        ps1 = ctx.enter_context(tc.tile_pool(name="ps1", bufs=2, space="PSUM"))
        ps2 = ctx.enter_context(tc.tile_pool(name="ps2", bufs=2, space="PSUM"))
        ps3 = ctx.enter_context(tc.tile_pool(name="ps3", bufs=1, space="PSUM"))
        pst_pool = ctx.enter_context(tc.tile_pool(name="pst", bufs=1, space="PSUM"))

        # window 0's token DMA is issued before the singles so the large
        # transfer overlaps the parameter loads instead of queuing behind them
        xw0 = xin.tile([128, WIN], BF16, tag="xw")
        nc.sync.dma_start(out=xw0[:, : min(WIN, t_pad)], in_=xT[:, : min(WIN, t_pad)])

        w1s = singles.tile([128, D_H], BF16)
        nc.sync.dma_start(out=w1s[:], in_=w1[:])
        w2s = singles.tile([128, 2 * D_OUT], BF16)
        nc.sync.dma_start(out=w2s[:], in_=w2[:])
        b1s = singles.tile([128, 2], F32)
        nc.sync.dma_start(out=b1s[:], in_=b1[:])
        b2s = singles.tile([128, 1], F32)
        nc.sync.dma_start(out=b2s[:], in_=b2[:])
        eyes = singles.tile([128, 128], F32)
        nc.sync.dma_start(out=eyes[:], in_=eye[:])
        gis = singles.tile([128, n_win * idxp], I16)
        nc.sync.dma_start(out=gis[:], in_=gidx[:])
        ics = singles.tile([128, n_tr], F32)
        nc.sync.dma_start(out=ics[:], in_=invc[:])
        ones = singles.tile([128, WIN], BF16)
        nc.gpsimd.memset(ones[:], 1.0)

        gpt = gp.tile([128, 1 + g_len], F32, tag="gpad")
        nc.gpsimd.memset(gpt[:], 0.0)
        sink = singles.tile([128, 1], F32)

        def emit_tile_epilogue(t):
            """Difference 128 slots, fix window-boundary slots, transpose to
            segment-major, scale by 1/count, DMA out."""
            tt = totp.tile([128, 128], F32, tag="tot")
            nc.vector.tensor_tensor(
                out=tt[:],
                in0=gpt[:, 1 + t * 128 : 1 + (t + 1) * 128],
                in1=gpt[:, t * 128 : (t + 1) * 128],
                op=SUB,
            )
            # first slot of window wb crosses into window wb-1: its diff is
            # missing the tail of window wb-1 past its last segment end; add
            # window wb-1's total, which the gather deposited in the reserved
            # padding slot spw-2 of wb-1's block (gpt col 1 + wb*spw - 2)
            wb = (t * 128 + spw - 1) // spw
            while wb * spw < (t + 1) * 128 and wb < n_win:
                if wb >= 1:
                    col = wb * spw - t * 128
                    wcol = 1 + wb * spw - 2
                    nc.vector.tensor_tensor(
                        out=tt[:, col : col + 1],
                        in0=tt[:, col : col + 1],
                        in1=gpt[:, wcol : wcol + 1],
                        op=ADD,
                    )
                wb += 1
            pst = pst_pool.tile([128, 128], F32, tag="pst")
            nc.tensor.transpose(pst[:], tt[:], eyes[:])
            ot = outp.tile([128, 128], F32, tag="ot")
            nc.vector.tensor_scalar_mul(ot[:], pst[:], ics[:, t : t + 1])
            nc.sync.dma_start(out=out[t * 128 : (t + 1) * 128, :], in_=ot[:])

        def evac(dst, src, bias_ap, engine):
            """relu(psum + bias) -> sbuf."""
            if engine == "act":
                nc.scalar.activation(dst[:], src[:], RELU, bias=bias_ap)
            else:
                nc.vector.tensor_scalar(
                    out=dst[:],
                    in0=src[:],
                    scalar1=bias_ap,
                    scalar2=0.0,
                    op0=ADD,
                    op1=MAX,
                )

        # win col 0 is read by gather idx 0 only as "prefix before any token"
        # (= 0, window-0 empty segments) or by padding slots of end-less
        # trailing windows (inv=0, discarded). The scan never writes col 0, so
        # zero it once in each of the 3 rotating buffers and never touch it
        # again -- no per-window carry copy to stall the DVE FIFO on.
        win_bufs = []
        for _ in range(3):
            wt = winp.tile([128, 1 + WIN], F32, tag="win")
            nc.vector.memset(wt[:, 0:1], 0.0)
            win_bufs.append(wt)

        for _rep in range(reps):
          # timing-only outer repetition; each rep rewrites the same output
          h2w_hist = {}
          win_hist = {}

          def emit_mlp(w, wlen, tok_base, xw=None):
            if xw is None:
                xw = xin.tile([128, WIN], BF16, tag="xw")
                nc.sync.dma_start(
                    out=xw[:, :wlen], in_=xT[:, tok_base : tok_base + wlen]
                )
            h2w = h2wp.tile([128, WIN], BF16, tag="h2w")
            h2w_hist[w] = h2w
            n_it = (wlen + ITER - 1) // ITER
            # iterations processed in pairs: matmuls grouped by stationary
            # operand (4 weight loads per pair), psum tiles span the pair so
            # each relu/evac class is a single wide op (same per-partition
            # bias across a pair -- same feature half, different tokens)
            for pi, p0 in enumerate(range(0, n_it, 2)):
                c0 = p0 * ITER
                plen = min(2 * ITER, wlen - c0)
                subs = [(0, min(ITER, plen))]
                if plen > ITER:
                    subs.append((ITER, plen - ITER))
                h2_ps = ps3.tile([128, 2 * ITER], F32, tag="h2_ps")
                h1ps, h1sbs = [], []
                for o, ln in subs:
                    h1a_ps = ps1.tile([128, ITER], F32, tag="h1a_ps")
                    h1b_ps = ps2.tile([128, ITER], F32, tag="h1b_ps")
                    h1ps.append((h1a_ps, h1b_ps))
                for (o, ln), (pa, pb) in zip(subs, h1ps):
                    nc.tensor.matmul(
                        pa[:, :ln], w1s[:, 0:128],
                        xw[:, c0 + o : c0 + o + ln], start=True, stop=True,
                    )
                for (o, ln), (pa, pb) in zip(subs, h1ps):
                    nc.tensor.matmul(
                        pb[:, :ln], w1s[:, 128:256],
                        xw[:, c0 + o : c0 + o + ln], start=True, stop=True,
                    )
                for i, ((o, ln), (pa, pb)) in enumerate(zip(subs, h1ps)):
                    h1a = h1sb.tile([128, ITER], BF16, tag="h1a")
                    h1b = h1sb.tile([128, ITER], BF16, tag="h1b")
                    evac(h1a[:, :ln], pa[:, :ln], b1s[:, 0:1], "act")
                    h1b_eng = (
                        "act"
                        if (
                            H1B_ACT_EVERY
                            and w % H1B_ACT_EVERY == 0
                            and pi == 0
                            and i == 0
                        )
                        else "dve"
                    )
                    evac(h1b[:, :ln], pb[:, :ln], b1s[:, 1:2], h1b_eng)
                    h1sbs.append((h1a, h1b))
                for (o, ln), (h1a, h1b) in zip(subs, h1sbs):
                    nc.tensor.matmul(
                        h2_ps[:, o : o + ln], w2s[:, 0:128], h1a[:, :ln],
                        start=True, stop=False,
                    )
                for (o, ln), (h1a, h1b) in zip(subs, h1sbs):
                    nc.tensor.matmul(
                        h2_ps[:, o : o + ln], w2s[:, 128:256], h1b[:, :ln],
                        start=False, stop=True,
                    )
                evac(h2w[:, c0 : c0 + plen], h2_ps[:, :plen], b2s[:, 0:1], "act")

          def emit_scan(w, wlen):
            # independent window-local cumsum (initial=0): no serial chain
            # between windows. Cross-window carry is repaired by the
            # boundary fixup using the gathered window totals.
            h2w = h2w_hist.pop(w)
            if mode == "mlp":
                nc.vector.tensor_copy(out=sink[:], in_=h2w[:, 0:1])
                return
            win = win_bufs[w % 3]
            win_hist[w] = win
            nc.vector.tensor_tensor_scan(
                out=win[:, 1 : 1 + wlen],
                data0=ones[:, :wlen],
                data1=h2w[:, :wlen],
                initial=0.0,
                op0=MULT,
                op1=ADD,
            )

          def emit_gather(w, wlen):
            if mode == "mlp":
                return
            win = win_hist.pop(w)
            if mode == "scan":
                nc.vector.tensor_copy(out=sink[:], in_=win[:, wlen : wlen + 1])
                return
            nc.gpsimd.ap_gather(
                out_ap=gpt[:, 1 + w * spw : 1 + (w + 1) * spw],
                in_ap=win[:, 0 : 1 + wlen],
                idxs_ap=gis[:, w * idxp : w * idxp + spw16],
                channels=128,
                num_elems=1 + wlen,
                d=1,
                num_idxs=spw,
            )

          # software-pipelined emission: window w's MLP, then the scan for
          # w-1, then the gather for w-2. Every op entering an engine FIFO has
          # dependencies at least a window old, and on Pool the chain-critical
          # scan is queued ahead of the (stale) gather so the DVE->Pool->DVE
          # scan chain never waits behind a 3us gather.
          done_tiles = 0
          tok_base = 0
          for w, wlen in enumerate(wlens):
            emit_mlp(w, wlen, tok_base, xw=xw0 if (w == 0 and _rep == 0) else None)
            tok_base += wlen
            if w >= 1:
                emit_scan(w - 1, wlens[w - 1])
            if w >= 2:
                emit_gather(w - 2, wlens[w - 2])
                avail = ((w - 1) * spw) // 128
                while mode == "full" and done_tiles < min(avail, n_tr):
                    emit_tile_epilogue(done_tiles)
                    done_tiles += 1
          emit_scan(n_win - 1, wlens[n_win - 1])
          if n_win >= 2:
            emit_gather(n_win - 2, wlens[n_win - 2])
          emit_gather(n_win - 1, wlens[n_win - 1])
          while mode == "full" and done_tiles < n_tr:
            emit_tile_epilogue(done_tiles)
            done_tiles += 1

    nc.compile()
    return nc


def _prepare(x, segment_ids, num_segments):
    """Host-side sharding + gather-index construction. Returns per-core
    metadata and the program size parameters."""
    T_total = x.shape[0]
    n_seg = int(num_segments)
    seg = np.asarray(segment_ids).astype(np.int64)
    counts = np.bincount(seg, minlength=n_seg).astype(np.int64)
    assert counts.max() < WIN, "segment longer than scan window unsupported"
    cum = np.cumsum(counts)

    # whole-segment split balanced by token count
    split = [0]
    for c in range(1, N_CORES):
        target = c * T_total / N_CORES
        s = int(np.searchsorted(cum, target))
        if s + 1 < n_seg and abs(cum[s] - target) < abs(
            (cum[s - 1] if s > 0 else 0) - target
        ):
            s = s + 1
        s = max(split[-1], min(s, n_seg))
        split.append(s)
    split.append(n_seg)

    cores = []
    max_tok = 1
    for c in range(N_CORES):
        s0, s1 = split[c], split[c + 1]
        t0 = int(cum[s0 - 1]) if s0 > 0 else 0
        t1 = int(cum[s1 - 1]) if s1 > 0 else 0
        cores.append({"s0": s0, "s1": s1, "t0": t0, "t1": t1})
        max_tok = max(max_tok, t1 - t0)

    # pad to 16 tokens (32B bf16 DMA alignment); partial tail window
    t_pad = int(math.ceil(max_tok / 16) * 16)
    wlens = _window_lengths(t_pad)
    n_win = len(wlens)

    # per-core per-window segment-end indices
    max_ends = 1
    for core in cores:
        s0, s1, t0 = core["s0"], core["s1"], core["t0"]
        ends = cum[s0:s1] - 1 - t0  # local end col per segment; may be -1
        win_of = np.minimum(np.maximum(ends, 0) // WIN, n_win - 1)
        idx_rel = ends - win_of * WIN + 1  # in [0, wlen]
        core["win_of"] = win_of
        core["idx_rel"] = idx_rel
        if len(ends):
            bc = np.bincount(win_of, minlength=n_win)
            max_ends = max(max_ends, int(bc.max()))

    # +2: slot spw-2 of each window is reserved to gather the window total
    # (scan output at col wlen) for the cross-window boundary fixup; slot
    # spw-1 must stay a repeat of the last real end for the boundary diff
    spw = int(math.ceil((max_ends + 2) / 16) * 16)
    n_tr = int(math.ceil(n_win * spw / 128))

    for core in cores:
        s0, s1 = core["s0"], core["s1"]
        n_loc = s1 - s0
        slot_of = np.zeros(n_loc, dtype=np.int64)
        idx_full = np.zeros(n_win * spw, dtype=np.int16)
        pos = np.zeros(n_win, dtype=np.int64)
        for j in range(n_loc):
            w = int(core["win_of"][j])
            k = int(pos[w])
            assert k < spw
            idx_full[w * spw + k] = core["idx_rel"][j]
            slot_of[j] = w * spw + k
            pos[w] = k + 1
        # pad each window by repeating its last real index. A window holding
        # real tokens always has >=1 end (counts.max() < WIN); end-less
        # windows are trailing ones past this core's token count, whose
        # slots are all padding (inv=0, never read by the host).
        n_tok_loc = core["t1"] - core["t0"]
        for w in range(n_win):
            k = int(pos[w])
            assert k > 0 or w * WIN >= n_tok_loc, "end-less window with tokens"
            assert k <= spw - 2
            last = idx_full[w * spw + k - 1] if k > 0 else np.int16(0)
            idx_full[w * spw + k : (w + 1) * spw] = last
            # reserved slot: gather this window's total for the boundary fixup
            idx_full[w * spw + spw - 2] = wlens[w]
        core["slot_of"] = slot_of
        # wrap for ap_gather: unwrapped[j] = idxs[j % 16, j // 16] per window,
        # each block padded to a 16B-aligned width
        idxp = ((spw // 16 + 7) // 8) * 8
        blocks = []
        for w in range(n_win):
            arr = idx_full[w * spw : (w + 1) * spw]
            blk = np.zeros((16, idxp), dtype=np.int16)
            blk[:, : spw // 16] = arr.reshape(spw // 16, 16).T
            blocks.append(blk)
        gidx16 = np.concatenate(blocks, axis=1)  # [16, n_win * idxp]
        core["gidx"] = np.tile(gidx16, (8, 1)).astype(np.int16)  # [128, ...]
        counts_loc = np.diff(np.concatenate([[0], cum]))[s0:s1] if s1 > s0 else []
        inv_slot = np.zeros(n_tr * 128, dtype=np.float32)
        if s1 > s0:
            inv_slot[slot_of] = 1.0 / np.maximum(counts[s0:s1], 1)
        core["invc"] = np.ascontiguousarray(
            inv_slot.reshape(n_tr, 128).T
        )  # [128, n_tr]

    return cores, t_pad, spw, n_tr


def _make_in_maps(cores, t_pad, x, W1, b1, W2, b2):
    w2_np = np.ascontiguousarray(
        np.concatenate([W2[:128, :], W2[128:, :]], axis=1)
    ).astype(ml_dtypes.bfloat16)
    w1_np = np.ascontiguousarray(W1).astype(ml_dtypes.bfloat16)
    b1_np = np.ascontiguousarray(np.stack([b1[:128], b1[128:]], axis=1))
    b2_np = np.ascontiguousarray(b2[:, None])
    eye_np = np.eye(128, dtype=np.float32)
    in_maps = []
    for core in cores:
        t0, t1 = core["t0"], core["t1"]
        xT_c = np.zeros((D_IN, t_pad), dtype=ml_dtypes.bfloat16)
        xT_c[:, : t1 - t0] = x[t0:t1].T.astype(ml_dtypes.bfloat16)
        in_maps.append(
            {
                "xT": xT_c,
                "w1": w1_np,
                "w2": w2_np,
                "b1": b1_np,
                "b2": b2_np,
                "eye": eye_np,
                "gidx": core["gidx"],
                "invc": core["invc"],
            }
        )
    return in_maps


_PROGRAM_CACHE = {}


def kernel(x, segment_ids, num_segments, W1, b1, W2, b2):
    x = np.ascontiguousarray(np.asarray(x, dtype=np.float32))
    W1 = np.asarray(W1, dtype=np.float32)
    b1 = np.asarray(b1, dtype=np.float32)
    W2 = np.asarray(W2, dtype=np.float32)
    b2 = np.asarray(b2, dtype=np.float32)
    n_seg = int(num_segments)

    cores, t_pad, spw, n_tr = _prepare(x, segment_ids, num_segments)

    key = (t_pad, spw, n_tr)
    if key not in _PROGRAM_CACHE:
        _PROGRAM_CACHE[key] = _build_program(t_pad, spw, n_tr)
    nc = _PROGRAM_CACHE[key]

    in_maps = _make_in_maps(cores, t_pad, x, W1, b1, W2, b2)
    res = run_bass_kernel_spmd(nc, in_maps, list(range(N_CORES)))

    out_full = np.zeros((n_seg, D_OUT), dtype=np.float32)
    for c, core in enumerate(cores):
        s0, s1 = core["s0"], core["s1"]
        if s1 > s0:
            out_full[s0:s1] = res.results[c]["out"][core["slot_of"]]
    return out_full
